# revision 8
# baseline (speedup 1.0000x reference)
"""Trainium2 Bass kernel for soft-MoE routing:
    gatings = softmax(x @ gw + gb, axis=1)            # [B, L]
    proj    = einsum('bi,oil->bol', x, pw)            # [B, D_OUT, L]
    result  = einsum('bol,bl->bo', proj, gatings) + gatings @ pb.T

Strategy (data-parallel over batch, 8 NeuronCores, 512 rows each):
  result[b,o] = ( sum_l E[b,l] * (x @ W_l^T + pb_l)[b,o] ) / sum_l E[b,l]
  with E = exp(x@gw + gb) (unnormalized; normalization folded into a final
  elementwise scale on host). Per core, everything is computed transposed
  ([out, b]) so the contraction dim (d_in) sits on SBUF partitions:
    - logits^T accumulated per-chunk as x^T chunks stream in (8 bf16 MMs)
    - E^T = exp(logits^T + gb) on ScalarE, stored bf16
    - per-leaf row scatter of E^T + GpSimd partition-broadcast
    - xg^T[i,b] = x^T[i,b] * E^T[l,b] on VectorE (bf16 in/out, 2x rate)
    - out^T[oc] += pw^T[l,i,oc-block] (stationary) x xg^T (moving), one long
      PSUM accumulation per 128-row output chunk (8 banks, 257 matmuls each)
    - final evacuation + DMA out; host divides by the E-sum.
  All main matmuls are bf16 (1 PE cycle/row; fp8 DoubleRow was measured to
  break the 2e-2 accuracy budget: e4m3 noise simulates to ~3.8e-2).
  Head is pipelined: x ships as bf16 (half the DMA), gating matmuls fire
  per-chunk, GpSimd runs no DMAs so the PE warmup starts immediately and
  the HAM clock-gate is open before the main stream begins.
"""
import numpy as np

B, D_IN, D_OUT, L = 4096, 1024, 1024, 32
NCORES = 8
P = 128                 # SBUF partitions
BSH = B // NCORES       # 512 batch rows per core
IC = D_IN // P          # 8 contraction chunks
OC = D_OUT // P         # 8 output chunks

_RUNNER = None


def _build_module():
    import concourse.mybir as mybir
    import concourse.tile as tile
    from concourse import bacc
    from concourse.bass import ts

    F32 = mybir.dt.float32
    BF16 = mybir.dt.bfloat16
    AFT = mybir.ActivationFunctionType

    nc = bacc.Bacc("TRN2", target_bir_lowering=False, debug=False)

    xt = nc.dram_tensor("xt", [D_IN, BSH], BF16, kind="ExternalInput")
    pwt = nc.dram_tensor("pwt", [L, D_IN, D_OUT], BF16, kind="ExternalInput")
    gwt_d = nc.dram_tensor("gwt", [D_IN, L], BF16, kind="ExternalInput")
    gb_d = nc.dram_tensor("gb", [L, 1], F32, kind="ExternalInput")
    outt = nc.dram_tensor("outt", [D_OUT, BSH], F32, kind="ExternalOutput")
    et_d = nc.dram_tensor("ets", [L, BSH], BF16, kind="ExternalOutput")

    NXP = 4                 # x^T ships in 4 two-chunk DMAs
    with tile.TileContext(nc) as tc:
        with tc.tile_pool(name="const", bufs=1) as cp:
            # one tile per x^T chunk-pair so each gating matmul depends only
            # on its own pair's DMA
            xps = [
                cp.tile([P, 2 * BSH], BF16, tag=f"xp{j}", name=f"xp{j}")
                for j in range(NXP)
            ]
            gwt = cp.tile([P, IC * L], BF16, tag="gwt")
            gbt = cp.tile([L, 1], F32, tag="gbt")
            et = cp.tile([L, BSH], BF16, tag="et")
            # one dedicated [1, BSH] tile per leaf: the HW partition-broadcast
            # ucode uses the tile base address (AP offsets are not honored),
            # so each leaf row needs its own tile.
            els = [
                cp.tile([1, BSH], BF16, tag=f"el{l}", name=f"el{l}")
                for l in range(L)
            ]
            warm_w = cp.tile([L, BSH], BF16, tag="warm_w")
            scratch = cp.tile([L, 1], F32, tag="scratch")

            def xchunk(c):
                return xps[c // 2][:, ts(c % 2, BSH)]

            # GpSimd runs no DMAs: its first op seeds the warmup weights so
            # the PE warm-up starts as soon as the engines are up.
            nc.gpsimd.memset(warm_w[:], 1.0)

            # input DMAs: everything gating-critical on the sync queue (the
            # weight stream lives on the scalar queue so it cannot delay
            # these), in chunk order
            with tc.high_priority():
                nc.sync.dma_start(
                    gwt[:].rearrange("p (c l) -> p c l", l=L),
                    gwt_d.rearrange("(c p) l -> p c l", p=P),
                )
                nc.scalar.dma_start(gbt[:], gb_d[:])
                for j in range(NXP):
                    nc.sync.dma_start(
                        xps[j][:].rearrange("p (two b) -> p two b", two=2),
                        xt[ts(j, 2 * P), :].rearrange("(two p) b -> p two b", p=P),
                    )

            # Preload the Exp activation table off the critical path.
            nc.scalar.activation(scratch[:], gbt[:], AFT.Exp, bias=0.0, scale=1.0)

            with tc.tile_pool(name="hpsum", bufs=4, space="PSUM") as hp:
                # PE warm-up while inputs DMA in: enough sustained busy-time
                # to open the HAM clock-gate before the real work.
                pw_ps = hp.tile([P, BSH], F32, tag="warm")
                for _ in range(4):
                    nc.tensor.matmul(pw_ps[:], warm_w[:, :P], warm_w[:],
                                     start=True, stop=True)
                # ---- gating head, pipelined with the x^T DMA ----
                pg = hp.tile([L, BSH], F32, tag="hp")
                for c in range(IC):
                    nc.tensor.matmul(
                        pg[:],
                        gwt[:, ts(c, L)],
                        xchunk(c),
                        start=(c == 0),
                        stop=(c == IC - 1),
                    )
                # E^T = exp(logits^T + gb); the sum over leaves (denominator)
                # and the pb bias term are computed on host from ets.
                nc.scalar.activation(et[:], pg[:], AFT.Exp, bias=gbt[:], scale=1.0)
                nc.sync.dma_start(et_d[:], et[:])

                # scatter E^T rows into the dedicated per-leaf tiles (tiny
                # bf16 DMAs on the non-weight-stream queue)
                for l in range(L):
                    nc.sync.dma_start(els[l][:], et[l:l + 1, :])

                # keep the PE busy (HAM stays un-throttled) while the first
                # leaf's scatter/broadcast/multiply chain completes
                for _ in range(10):
                    nc.tensor.matmul(pw_ps[:, :256], warm_w[:, :P],
                                     warm_w[:, :256], start=True, stop=True)

            # ---- main accumulation ----
            with tc.tile_pool(name="opsum", bufs=8, space="PSUM") as op, \
                 tc.tile_pool(name="wpool", bufs=12) as wp, \
                 tc.tile_pool(name="xgpool", bufs=10) as xp, \
                 tc.tile_pool(name="ebcpool", bufs=4) as bp, \
                 tc.tile_pool(name="evac", bufs=4) as ep:
                pos = [
                    op.tile([P, BSH], F32, tag="po", name=f"po{oc}")
                    for oc in range(OC)
                ]
                def evacuate(oc):
                    # unnormalized sums out (normalization happens on host);
                    # copies alternate ScalarE/VectorE
                    ot = ep.tile([P, BSH], F32, tag="ot", name=f"ot{oc}")
                    if oc % 2 == 0:
                        nc.scalar.copy(ot[:], pos[oc][:])
                        nc.sync.dma_start(outt[ts(oc, P), :], ot[:])
                    else:
                        nc.vector.tensor_copy(ot[:], pos[oc][:])
                        nc.scalar.dma_start(outt[ts(oc, P), :], ot[:])

                for l in range(L - 1):
                    # broadcast this leaf's gates across partitions on GpSimd
                    ebc = bp.tile([P, BSH], BF16, tag="ebc")
                    nc.gpsimd.partition_broadcast(ebc[:], els[l][:])
                    for c in range(IC):
                        wt = wp.tile([P, D_OUT], BF16, tag="wt")
                        nc.scalar.dma_start(wt[:], pwt[l, ts(c, P), :])
                        xg = xp.tile([P, BSH], BF16, tag="xg")
                        nc.vector.tensor_mul(
                            xg[:], xchunk(c), ebc[:]
                        )
                        for oc in range(OC):
                            nc.tensor.matmul(
                                pos[oc][:], wt[:, ts(oc, P)], xg[:],
                                start=(l == 0 and c == 0), stop=False,
                            )
                # Last leaf: bank-at-a-time so 7 of 8 banks finish early and
                # their evacuation + output DMAs overlap the remaining matmuls.
                l = L - 1
                ebc = bp.tile([P, BSH], BF16, tag="ebc")
                nc.gpsimd.partition_broadcast(ebc[:], els[l][:])
                wts, xgs = [], []
                for c in range(IC):
                    wt = wp.tile([P, D_OUT], BF16, tag="wt", name=f"wtl{c}")
                    nc.scalar.dma_start(wt[:], pwt[l, ts(c, P), :])
                    wts.append(wt)
                    xg = xp.tile([P, BSH], BF16, tag="xg", name=f"xgl{c}")
                    nc.vector.tensor_mul(xg[:], xchunk(c), ebc[:])
                    xgs.append(xg)
                for oc in range(OC):
                    for c in range(IC):
                        nc.tensor.matmul(
                            pos[oc][:], wts[c][:, ts(oc, P)], xgs[c][:],
                            start=False, stop=(c == IC - 1),
                        )
                    evacuate(oc)

    nc.compile()
    return nc


def _make_runner(nc):
    """Cached shard_map-jitted executor over 8 cores (mirrors
    concourse.bass2jax.run_bass_via_pjrt, but reusable across calls)."""
    import jax
    import numpy as np
    from jax.sharding import Mesh, PartitionSpec
    from jax.experimental.shard_map import shard_map
    import concourse.mybir as mybir
    from concourse.bass2jax import (
        _bass_exec_p,
        install_neuronx_cc_hook,
        partition_id_tensor,
    )

    install_neuronx_cc_hook()

    partition_name = (
        nc.partition_id_tensor.name if nc.partition_id_tensor else None
    )
    in_names, out_names, out_avals, zero_shapes = [], [], [], []
    for alloc in nc.m.functions[0].allocations:
        if not isinstance(alloc, mybir.MemoryLocationSet):
            continue
        name = alloc.memorylocations[0].name
        if alloc.kind == "ExternalInput":
            if name != partition_name:
                in_names.append(name)
        elif alloc.kind == "ExternalOutput":
            shape = tuple(alloc.tensor_shape)
            dtype = mybir.dt.np(alloc.dtype)
            out_avals.append(jax.core.ShapedArray(shape, dtype))
            zero_shapes.append((shape, dtype))
            out_names.append(name)
    n_params = len(in_names)
    n_outs = len(out_avals)
    all_names = tuple(in_names + out_names)
    if partition_name is not None:
        all_names = all_names + (partition_name,)
    donate = tuple(range(n_params, n_params + n_outs))

    def _body(*args):
        operands = list(args)
        if partition_name is not None:
            operands.append(partition_id_tensor())
        outs = _bass_exec_p.bind(
            *operands,
            out_avals=tuple(out_avals),
            in_names=all_names,
            out_names=tuple(out_names),
            lowering_input_output_aliases=(),
            sim_require_finite=True,
            sim_require_nnan=True,
            nc=nc,
        )
        return tuple(outs)

    devices = jax.devices()[:NCORES]
    mesh = Mesh(np.asarray(devices), ("core",))
    sharded = jax.jit(
        shard_map(
            _body,
            mesh=mesh,
            in_specs=(PartitionSpec("core"),) * (n_params + n_outs),
            out_specs=(PartitionSpec("core"),) * n_outs,
            check_rep=False,
        ),
        donate_argnums=donate,
        keep_unused=True,
    )

    def run(in_maps):
        concat_in = [
            np.concatenate([m[name] for m in in_maps], axis=0)
            for name in in_names
        ]
        concat_zeros = [
            np.zeros((NCORES * s[0], *s[1:]), dt) for s, dt in zero_shapes
        ]
        out_arrs = sharded(*concat_in, *concat_zeros)
        return [
            {
                name: np.asarray(out_arrs[i]).reshape(
                    NCORES, *out_avals[i].shape
                )[c]
                for i, name in enumerate(out_names)
            }
            for c in range(NCORES)
        ]

    return run


def make_in_maps(x, gw, gb, pw, pb):
    """Shard + lay out the full inputs into per-core input maps."""
    import ml_dtypes
    bf = ml_dtypes.bfloat16
    pwt = np.ascontiguousarray(
        pw.transpose(2, 1, 0).astype(bf))                             # [L, D_IN, D_OUT]
    gwr = np.ascontiguousarray(np.asarray(gw, np.float32).astype(bf))
    gbc = np.ascontiguousarray(gb, dtype=np.float32).reshape(L, 1)
    in_maps = []
    for c in range(NCORES):
        xtc = np.ascontiguousarray(
            np.asarray(x[c * BSH:(c + 1) * BSH, :], np.float32).T.astype(bf))
        in_maps.append({"xt": xtc, "pwt": pwt, "gwt": gwr, "gb": gbc})
    return in_maps


def finish_host(results, pb):
    """Normalize by the gate sum and add the host-side pb bias term."""
    pbf = np.asarray(pb, np.float32)                  # [D_OUT, L]
    outs = []
    for r in results:
        et = r["ets"].astype(np.float32)              # [L, BSH] unnormalized E
        den = et.sum(axis=0)                          # [BSH]
        g = (et / den).T                              # [BSH, L] gatings
        outs.append(r["outt"].T / den[:, None] + g @ pbf.T)
    return np.ascontiguousarray(np.concatenate(outs, axis=0), dtype=np.float32)


def _get_runner():
    global _RUNNER
    if _RUNNER is None:
        nc = _build_module()
        try:
            _RUNNER = _make_runner(nc)
        except Exception:
            # Fallback: the (slower, non-cached) stock execution path.
            from concourse.bass_utils import run_bass_kernel_spmd

            def _run(in_maps):
                return run_bass_kernel_spmd(
                    nc, in_maps, core_ids=list(range(NCORES))
                ).results

            _RUNNER = _run
    return _RUNNER


def kernel(x, gw, gb, pw, pb):
    global _RUNNER
    in_maps = make_in_maps(x, gw, gb, pw, pb)
    try:
        results = _get_runner()(in_maps)
    except Exception:
        # One retry with a freshly built runner (e.g. transient device error).
        _RUNNER = None
        results = _get_runner()(in_maps)
    return finish_host(results, pb)


# revision 10
# speedup vs baseline: 1.0060x; 1.0060x over previous
"""Trainium2 Bass kernel for soft-MoE routing:
    gatings = softmax(x @ gw + gb, axis=1)            # [B, L]
    proj    = einsum('bi,oil->bol', x, pw)            # [B, D_OUT, L]
    result  = einsum('bol,bl->bo', proj, gatings) + gatings @ pb.T

Strategy (data-parallel over batch, 8 NeuronCores, 512 rows each):
  result[b,o] = ( sum_l E[b,l] * (x @ W_l^T + pb_l)[b,o] ) / sum_l E[b,l]
  with E = exp(x@gw + gb) (unnormalized; normalization folded into a final
  elementwise scale on host). Per core, everything is computed transposed
  ([out, b]) so the contraction dim (d_in) sits on SBUF partitions:
    - logits^T accumulated per-chunk as x^T chunks stream in (8 bf16 MMs)
    - E^T = exp(logits^T + gb) on ScalarE, stored bf16
    - per-leaf row scatter of E^T + GpSimd partition-broadcast
    - xg^T[i,b] = x^T[i,b] * E^T[l,b] on VectorE (bf16 in/out, 2x rate)
    - out^T[oc] += pw^T[l,i,oc-block] (stationary) x xg^T (moving), one long
      PSUM accumulation per 128-row output chunk (8 banks, 257 matmuls each)
    - final evacuation + DMA out; host divides by the E-sum.
  All main matmuls are bf16 (1 PE cycle/row; fp8 DoubleRow was measured to
  break the 2e-2 accuracy budget: e4m3 noise simulates to ~3.8e-2).
  Head is pipelined: x ships as bf16 (half the DMA), gating matmuls fire
  per-chunk, GpSimd runs no DMAs so the PE warmup starts immediately and
  the HAM clock-gate is open before the main stream begins.
"""
import numpy as np

B, D_IN, D_OUT, L = 4096, 1024, 1024, 32
NCORES = 8
P = 128                 # SBUF partitions
BSH = B // NCORES       # 512 batch rows per core
IC = D_IN // P          # 8 contraction chunks
OC = D_OUT // P         # 8 output chunks

_RUNNER = None


def _build_module():
    import concourse.mybir as mybir
    import concourse.tile as tile
    from concourse import bacc
    from concourse.bass import ts

    F32 = mybir.dt.float32
    BF16 = mybir.dt.bfloat16
    AFT = mybir.ActivationFunctionType

    nc = bacc.Bacc("TRN2", target_bir_lowering=False, debug=False)

    xt = nc.dram_tensor("xt", [D_IN, BSH], BF16, kind="ExternalInput")
    pwt = nc.dram_tensor("pwt", [L, D_IN, D_OUT], BF16, kind="ExternalInput")
    gwt_d = nc.dram_tensor("gwt", [D_IN, L], BF16, kind="ExternalInput")
    gb_d = nc.dram_tensor("gb", [L, 1], F32, kind="ExternalInput")
    outt = nc.dram_tensor("outt", [D_OUT, BSH], F32, kind="ExternalOutput")
    et_d = nc.dram_tensor("ets", [L, BSH], BF16, kind="ExternalOutput")

    with tile.TileContext(nc) as tc:
        with tc.tile_pool(name="const", bufs=1) as cp:
            # one tile per x^T chunk so each gating matmul depends only on
            # its own chunk's DMA
            xts = [
                cp.tile([P, BSH], BF16, tag=f"xt{c}", name=f"xt{c}")
                for c in range(IC)
            ]
            gwt = cp.tile([P, IC * L], BF16, tag="gwt")
            gbt = cp.tile([L, 1], F32, tag="gbt")
            et = cp.tile([L, BSH], BF16, tag="et")
            # one dedicated [1, BSH] tile per leaf: the HW partition-broadcast
            # ucode uses the tile base address (AP offsets are not honored),
            # so each leaf row needs its own tile.
            els = [
                cp.tile([1, BSH], BF16, tag=f"el{l}", name=f"el{l}")
                for l in range(L)
            ]
            warm_w = cp.tile([L, 256], BF16, tag="warm_w")
            scratch = cp.tile([L, 1], F32, tag="scratch")
            dummy = cp.tile([L, 1], F32, tag="dummy")

            def xchunk(c):
                return xts[c][:]

            # GpSimd runs no DMAs: its first op seeds the warmup weights so
            # the PE warm-up starts as soon as the engines are up.
            nc.gpsimd.memset(warm_w[:], 1.0)

            # tiny dummy DMAs first: prime the DMA rings + completion path
            # so the real input DMAs' semaphores deliver promptly
            nc.sync.dma_start(dummy[:], gb_d[:])
            nc.scalar.dma_start(gbt[:], gb_d[:])

            # input DMAs: x^T chunks interleaved across both HWDGE queues in
            # chunk order, gating weights first
            nc.sync.dma_start(
                gwt[:].rearrange("p (c l) -> p c l", l=L),
                gwt_d.rearrange("(c p) l -> p c l", p=P),
            )
            for c in range(IC):
                eng = nc.sync if c % 2 == 0 else nc.scalar
                eng.dma_start(xchunk(c), xt[ts(c, P), :])

            # Preload the Exp activation table off the critical path.
            nc.scalar.activation(scratch[:], gbt[:], AFT.Exp, bias=0.0, scale=1.0)

            with tc.tile_pool(name="hpsum", bufs=4, space="PSUM") as hp:
                # PE warm-up while inputs DMA in: enough sustained busy-time
                # to open the HAM clock-gate before the real work.
                pw_ps = hp.tile([P, 256], F32, tag="warm")
                for _ in range(12):
                    nc.tensor.matmul(pw_ps[:], warm_w[:, :P], warm_w[:],
                                     start=True, stop=True)
                # ---- gating head, pipelined with the x^T DMA ----
                pg = hp.tile([L, BSH], F32, tag="hp")
                for c in range(IC):
                    nc.tensor.matmul(
                        pg[:],
                        gwt[:, ts(c, L)],
                        xchunk(c),
                        start=(c == 0),
                        stop=(c == IC - 1),
                    )
                # E^T = exp(logits^T + gb); the sum over leaves (denominator)
                # and the pb bias term are computed on host from ets.
                nc.scalar.activation(et[:], pg[:], AFT.Exp, bias=gbt[:], scale=1.0)

                # scatter E^T rows into the dedicated per-leaf tiles (tiny
                # bf16 DMAs), then ship E^T to the host
                for l in range(L):
                    nc.scalar.dma_start(els[l][:], et[l:l + 1, :])
                nc.scalar.dma_start(et_d[:], et[:])

                # keep the PE busy (HAM stays un-throttled) while the first
                # leaf's scatter/broadcast/multiply chain completes
                for _ in range(8):
                    nc.tensor.matmul(pw_ps[:], warm_w[:, :P],
                                     warm_w[:], start=True, stop=True)

            # ---- main accumulation ----
            with tc.tile_pool(name="opsum", bufs=8, space="PSUM") as op, \
                 tc.tile_pool(name="wpool", bufs=12) as wp, \
                 tc.tile_pool(name="xgpool", bufs=10) as xp, \
                 tc.tile_pool(name="ebcpool", bufs=4) as bp, \
                 tc.tile_pool(name="evac", bufs=4) as ep:
                pos = [
                    op.tile([P, BSH], F32, tag="po", name=f"po{oc}")
                    for oc in range(OC)
                ]
                def evacuate(oc):
                    # unnormalized sums out (normalization happens on host);
                    # copies alternate ScalarE/VectorE
                    ot = ep.tile([P, BSH], F32, tag="ot", name=f"ot{oc}")
                    if oc % 2 == 0:
                        nc.scalar.copy(ot[:], pos[oc][:])
                        nc.sync.dma_start(outt[ts(oc, P), :], ot[:])
                    else:
                        nc.vector.tensor_copy(ot[:], pos[oc][:])
                        nc.scalar.dma_start(outt[ts(oc, P), :], ot[:])

                for l in range(L - 1):
                    # broadcast this leaf's gates across partitions on GpSimd
                    ebc = bp.tile([P, BSH], BF16, tag="ebc")
                    nc.gpsimd.partition_broadcast(ebc[:], els[l][:])
                    for c in range(IC):
                        wt = wp.tile([P, D_OUT], BF16, tag="wt")
                        nc.sync.dma_start(wt[:], pwt[l, ts(c, P), :])
                        xg = xp.tile([P, BSH], BF16, tag="xg")
                        nc.vector.tensor_mul(
                            xg[:], xchunk(c), ebc[:]
                        )
                        for oc in range(OC):
                            nc.tensor.matmul(
                                pos[oc][:], wt[:, ts(oc, P)], xg[:],
                                start=(l == 0 and c == 0), stop=False,
                            )
                # Last leaf: bank-at-a-time so 7 of 8 banks finish early and
                # their evacuation + output DMAs overlap the remaining matmuls.
                l = L - 1
                ebc = bp.tile([P, BSH], BF16, tag="ebc")
                nc.gpsimd.partition_broadcast(ebc[:], els[l][:])
                wts, xgs = [], []
                for c in range(IC):
                    wt = wp.tile([P, D_OUT], BF16, tag="wt", name=f"wtl{c}")
                    nc.sync.dma_start(wt[:], pwt[l, ts(c, P), :])
                    wts.append(wt)
                    xg = xp.tile([P, BSH], BF16, tag="xg", name=f"xgl{c}")
                    nc.vector.tensor_mul(xg[:], xchunk(c), ebc[:])
                    xgs.append(xg)
                for oc in range(OC):
                    for c in range(IC):
                        nc.tensor.matmul(
                            pos[oc][:], wts[c][:, ts(oc, P)], xgs[c][:],
                            start=False, stop=(c == IC - 1),
                        )
                    evacuate(oc)

    nc.compile()
    return nc


def _make_runner(nc):
    """Cached shard_map-jitted executor over 8 cores (mirrors
    concourse.bass2jax.run_bass_via_pjrt, but reusable across calls)."""
    import jax
    import numpy as np
    from jax.sharding import Mesh, PartitionSpec
    from jax.experimental.shard_map import shard_map
    import concourse.mybir as mybir
    from concourse.bass2jax import (
        _bass_exec_p,
        install_neuronx_cc_hook,
        partition_id_tensor,
    )

    install_neuronx_cc_hook()

    partition_name = (
        nc.partition_id_tensor.name if nc.partition_id_tensor else None
    )
    in_names, out_names, out_avals, zero_shapes = [], [], [], []
    for alloc in nc.m.functions[0].allocations:
        if not isinstance(alloc, mybir.MemoryLocationSet):
            continue
        name = alloc.memorylocations[0].name
        if alloc.kind == "ExternalInput":
            if name != partition_name:
                in_names.append(name)
        elif alloc.kind == "ExternalOutput":
            shape = tuple(alloc.tensor_shape)
            dtype = mybir.dt.np(alloc.dtype)
            out_avals.append(jax.core.ShapedArray(shape, dtype))
            zero_shapes.append((shape, dtype))
            out_names.append(name)
    n_params = len(in_names)
    n_outs = len(out_avals)
    all_names = tuple(in_names + out_names)
    if partition_name is not None:
        all_names = all_names + (partition_name,)
    donate = tuple(range(n_params, n_params + n_outs))

    def _body(*args):
        operands = list(args)
        if partition_name is not None:
            operands.append(partition_id_tensor())
        outs = _bass_exec_p.bind(
            *operands,
            out_avals=tuple(out_avals),
            in_names=all_names,
            out_names=tuple(out_names),
            lowering_input_output_aliases=(),
            sim_require_finite=True,
            sim_require_nnan=True,
            nc=nc,
        )
        return tuple(outs)

    devices = jax.devices()[:NCORES]
    mesh = Mesh(np.asarray(devices), ("core",))
    sharded = jax.jit(
        shard_map(
            _body,
            mesh=mesh,
            in_specs=(PartitionSpec("core"),) * (n_params + n_outs),
            out_specs=(PartitionSpec("core"),) * n_outs,
            check_rep=False,
        ),
        donate_argnums=donate,
        keep_unused=True,
    )

    def run(in_maps):
        concat_in = [
            np.concatenate([m[name] for m in in_maps], axis=0)
            for name in in_names
        ]
        concat_zeros = [
            np.zeros((NCORES * s[0], *s[1:]), dt) for s, dt in zero_shapes
        ]
        out_arrs = sharded(*concat_in, *concat_zeros)
        return [
            {
                name: np.asarray(out_arrs[i]).reshape(
                    NCORES, *out_avals[i].shape
                )[c]
                for i, name in enumerate(out_names)
            }
            for c in range(NCORES)
        ]

    return run


def make_in_maps(x, gw, gb, pw, pb):
    """Shard + lay out the full inputs into per-core input maps."""
    import ml_dtypes
    bf = ml_dtypes.bfloat16
    pwt = np.ascontiguousarray(
        pw.transpose(2, 1, 0).astype(bf))                             # [L, D_IN, D_OUT]
    gwr = np.ascontiguousarray(np.asarray(gw, np.float32).astype(bf))
    gbc = np.ascontiguousarray(gb, dtype=np.float32).reshape(L, 1)
    in_maps = []
    for c in range(NCORES):
        xtc = np.ascontiguousarray(
            np.asarray(x[c * BSH:(c + 1) * BSH, :], np.float32).T.astype(bf))
        in_maps.append({"xt": xtc, "pwt": pwt, "gwt": gwr, "gb": gbc})
    return in_maps


def finish_host(results, pb):
    """Normalize by the gate sum and add the host-side pb bias term."""
    pbf = np.asarray(pb, np.float32)                  # [D_OUT, L]
    outs = []
    for r in results:
        et = r["ets"].astype(np.float32)              # [L, BSH] unnormalized E
        den = et.sum(axis=0)                          # [BSH]
        g = (et / den).T                              # [BSH, L] gatings
        outs.append(r["outt"].T / den[:, None] + g @ pbf.T)
    return np.ascontiguousarray(np.concatenate(outs, axis=0), dtype=np.float32)


def _get_runner():
    global _RUNNER
    if _RUNNER is None:
        nc = _build_module()
        try:
            _RUNNER = _make_runner(nc)
        except Exception:
            # Fallback: the (slower, non-cached) stock execution path.
            from concourse.bass_utils import run_bass_kernel_spmd

            def _run(in_maps):
                return run_bass_kernel_spmd(
                    nc, in_maps, core_ids=list(range(NCORES))
                ).results

            _RUNNER = _run
    return _RUNNER


def kernel(x, gw, gb, pw, pb):
    global _RUNNER
    in_maps = make_in_maps(x, gw, gb, pw, pb)
    try:
        results = _get_runner()(in_maps)
    except Exception:
        # One retry with a freshly built runner (e.g. transient device error).
        _RUNNER = None
        results = _get_runner()(in_maps)
    return finish_host(results, pb)


# revision 11
# speedup vs baseline: 1.0135x; 1.0074x over previous
"""Trainium2 Bass kernel for soft-MoE routing:
    gatings = softmax(x @ gw + gb, axis=1)            # [B, L]
    proj    = einsum('bi,oil->bol', x, pw)            # [B, D_OUT, L]
    result  = einsum('bol,bl->bo', proj, gatings) + gatings @ pb.T

Strategy (data-parallel over batch, 8 NeuronCores, 512 rows each):
  result[b,o] = ( sum_l E[b,l] * (x @ W_l^T + pb_l)[b,o] ) / sum_l E[b,l]
  with E = exp(x@gw + gb) (unnormalized; normalization folded into a final
  elementwise scale on host). Per core, everything is computed transposed
  ([out, b]) so the contraction dim (d_in) sits on SBUF partitions:
    - logits^T accumulated per-chunk as x^T chunks stream in (8 bf16 MMs)
    - E^T = exp(logits^T + gb) on ScalarE, stored bf16
    - per-leaf row scatter of E^T + GpSimd partition-broadcast
    - xg^T[i,b] = x^T[i,b] * E^T[l,b] on VectorE (bf16 in/out, 2x rate)
    - out^T[oc] += pw^T[l,i,oc-block] (stationary) x xg^T (moving), one long
      PSUM accumulation per 128-row output chunk (8 banks, 257 matmuls each)
    - final evacuation + DMA out; host divides by the E-sum.
  All main matmuls are bf16 (1 PE cycle/row; fp8 DoubleRow was measured to
  break the 2e-2 accuracy budget: e4m3 noise simulates to ~3.8e-2).
  Head is pipelined: x ships as bf16 (half the DMA), gating matmuls fire
  per-chunk, GpSimd runs no DMAs so the PE warmup starts immediately and
  the HAM clock-gate is open before the main stream begins.
"""
import numpy as np

B, D_IN, D_OUT, L = 4096, 1024, 1024, 32
NCORES = 8
P = 128                 # SBUF partitions
BSH = B // NCORES       # 512 batch rows per core
IC = D_IN // P          # 8 contraction chunks
OC = D_OUT // P         # 8 output chunks

_RUNNER = None


def _build_module():
    import concourse.mybir as mybir
    import concourse.tile as tile
    from concourse import bacc
    from concourse.bass import ts

    F32 = mybir.dt.float32
    BF16 = mybir.dt.bfloat16
    AFT = mybir.ActivationFunctionType

    nc = bacc.Bacc("TRN2", target_bir_lowering=False, debug=False)

    xt = nc.dram_tensor("xt", [D_IN, BSH], BF16, kind="ExternalInput")
    pwt = nc.dram_tensor("pwt", [L, D_IN, D_OUT], BF16, kind="ExternalInput")
    gwt_d = nc.dram_tensor("gwt", [D_IN, L], BF16, kind="ExternalInput")
    gb_d = nc.dram_tensor("gb", [L, 1], F32, kind="ExternalInput")
    outt = nc.dram_tensor("outt", [D_OUT, BSH], F32, kind="ExternalOutput")
    et_d = nc.dram_tensor("ets", [L, BSH], BF16, kind="ExternalOutput")

    with tile.TileContext(nc) as tc:
        with tc.tile_pool(name="const", bufs=1) as cp:
            # one tile per x^T chunk so each gating matmul depends only on
            # its own chunk's DMA
            xts = [
                cp.tile([P, BSH], BF16, tag=f"xt{c}", name=f"xt{c}")
                for c in range(IC)
            ]
            gwt = cp.tile([P, IC * L], BF16, tag="gwt")
            gbt = cp.tile([L, 1], F32, tag="gbt")
            et = cp.tile([L, BSH], BF16, tag="et")
            # one dedicated [1, BSH] tile per leaf: the HW partition-broadcast
            # ucode uses the tile base address (AP offsets are not honored),
            # so each leaf row needs its own tile.
            els = [
                cp.tile([1, BSH], BF16, tag=f"el{l}", name=f"el{l}")
                for l in range(L)
            ]
            warm_w = cp.tile([L, 256], BF16, tag="warm_w")
            warm_v = cp.tile([L, 256], BF16, tag="warm_v")
            warm_b = cp.tile([P, 128], BF16, tag="warm_b")
            scratch = cp.tile([L, 1], F32, tag="scratch")
            dummy = cp.tile([L, 1], F32, tag="dummy")

            def xchunk(c):
                return xts[c][:]

            # GpSimd runs no DMAs: its first op seeds the warmup weights so
            # the PE warm-up starts as soon as the engines are up.
            nc.gpsimd.memset(warm_w[:], 1.0)
            # warm the broadcast ucode + DVE multiply path off the critical
            # path so leaf 0's broadcast/multiply chain runs at full speed
            nc.gpsimd.partition_broadcast(warm_b[:], warm_w[0:1, :128])
            nc.vector.tensor_mul(warm_v[:], warm_w[:], warm_w[:])
            nc.vector.tensor_mul(warm_v[:], warm_w[:], warm_w[:])

            # tiny dummy DMAs first: prime the DMA rings + completion path
            # so the real input DMAs' semaphores deliver promptly
            nc.sync.dma_start(dummy[:], gb_d[:])
            nc.scalar.dma_start(gbt[:], gb_d[:])

            # input DMAs: x^T chunks interleaved across both HWDGE queues in
            # chunk order, gating weights first
            nc.sync.dma_start(
                gwt[:].rearrange("p (c l) -> p c l", l=L),
                gwt_d.rearrange("(c p) l -> p c l", p=P),
            )
            for c in range(IC):
                eng = nc.sync if c % 2 == 0 else nc.scalar
                eng.dma_start(xchunk(c), xt[ts(c, P), :])

            # Preload the Exp activation table off the critical path.
            nc.scalar.activation(scratch[:], gbt[:], AFT.Exp, bias=0.0, scale=1.0)

            with tc.tile_pool(name="hpsum", bufs=4, space="PSUM") as hp:
                # PE warm-up while inputs DMA in: enough sustained busy-time
                # to open the HAM clock-gate before the real work.
                pw_ps = hp.tile([P, 256], F32, tag="warm")
                for _ in range(12):
                    nc.tensor.matmul(pw_ps[:], warm_w[:, :P], warm_w[:],
                                     start=True, stop=True)
                # ---- gating head, pipelined with the x^T DMA ----
                pg = hp.tile([L, BSH], F32, tag="hp")
                for c in range(IC):
                    nc.tensor.matmul(
                        pg[:],
                        gwt[:, ts(c, L)],
                        xchunk(c),
                        start=(c == 0),
                        stop=(c == IC - 1),
                    )
                # E^T = exp(logits^T + gb); the sum over leaves (denominator)
                # and the pb bias term are computed on host from ets.
                nc.scalar.activation(et[:], pg[:], AFT.Exp, bias=gbt[:], scale=1.0)

                # scatter E^T rows into the dedicated per-leaf tiles (tiny
                # bf16 DMAs), then ship E^T to the host
                for l in range(L):
                    nc.scalar.dma_start(els[l][:], et[l:l + 1, :])
                nc.scalar.dma_start(et_d[:], et[:])

                # keep the PE busy (HAM stays un-throttled) while the first
                # leaf's scatter/broadcast/multiply chain completes
                for _ in range(8):
                    nc.tensor.matmul(pw_ps[:], warm_w[:, :P],
                                     warm_w[:], start=True, stop=True)

            # ---- main accumulation ----
            with tc.tile_pool(name="opsum", bufs=8, space="PSUM") as op, \
                 tc.tile_pool(name="wpool", bufs=12) as wp, \
                 tc.tile_pool(name="xgpool", bufs=10) as xp, \
                 tc.tile_pool(name="ebcpool", bufs=4) as bp, \
                 tc.tile_pool(name="evac", bufs=4) as ep:
                pos = [
                    op.tile([P, BSH], F32, tag="po", name=f"po{oc}")
                    for oc in range(OC)
                ]
                def evacuate(oc):
                    # unnormalized sums out (normalization happens on host);
                    # copies alternate ScalarE/VectorE
                    ot = ep.tile([P, BSH], F32, tag="ot", name=f"ot{oc}")
                    if oc % 2 == 0:
                        nc.scalar.copy(ot[:], pos[oc][:])
                        nc.sync.dma_start(outt[ts(oc, P), :], ot[:])
                    else:
                        nc.vector.tensor_copy(ot[:], pos[oc][:])
                        nc.scalar.dma_start(outt[ts(oc, P), :], ot[:])

                for l in range(L - 1):
                    # broadcast this leaf's gates across partitions on GpSimd
                    ebc = bp.tile([P, BSH], BF16, tag="ebc")
                    nc.gpsimd.partition_broadcast(ebc[:], els[l][:])
                    for c in range(IC):
                        wt = wp.tile([P, D_OUT], BF16, tag="wt")
                        nc.sync.dma_start(wt[:], pwt[l, ts(c, P), :])
                        xg = xp.tile([P, BSH], BF16, tag="xg")
                        nc.vector.tensor_mul(
                            xg[:], xchunk(c), ebc[:]
                        )
                        for oc in range(OC):
                            nc.tensor.matmul(
                                pos[oc][:], wt[:, ts(oc, P)], xg[:],
                                start=(l == 0 and c == 0), stop=False,
                            )
                # Last leaf: bank-at-a-time so 7 of 8 banks finish early and
                # their evacuation + output DMAs overlap the remaining matmuls.
                l = L - 1
                ebc = bp.tile([P, BSH], BF16, tag="ebc")
                nc.gpsimd.partition_broadcast(ebc[:], els[l][:])
                wts, xgs = [], []
                for c in range(IC):
                    wt = wp.tile([P, D_OUT], BF16, tag="wt", name=f"wtl{c}")
                    nc.sync.dma_start(wt[:], pwt[l, ts(c, P), :])
                    wts.append(wt)
                    xg = xp.tile([P, BSH], BF16, tag="xg", name=f"xgl{c}")
                    nc.vector.tensor_mul(xg[:], xchunk(c), ebc[:])
                    xgs.append(xg)
                for oc in range(OC):
                    for c in range(IC):
                        nc.tensor.matmul(
                            pos[oc][:], wts[c][:, ts(oc, P)], xgs[c][:],
                            start=False, stop=(c == IC - 1),
                        )
                    evacuate(oc)

    nc.compile()
    return nc


def _make_runner(nc):
    """Cached shard_map-jitted executor over 8 cores (mirrors
    concourse.bass2jax.run_bass_via_pjrt, but reusable across calls)."""
    import jax
    import numpy as np
    from jax.sharding import Mesh, PartitionSpec
    from jax.experimental.shard_map import shard_map
    import concourse.mybir as mybir
    from concourse.bass2jax import (
        _bass_exec_p,
        install_neuronx_cc_hook,
        partition_id_tensor,
    )

    install_neuronx_cc_hook()

    partition_name = (
        nc.partition_id_tensor.name if nc.partition_id_tensor else None
    )
    in_names, out_names, out_avals, zero_shapes = [], [], [], []
    for alloc in nc.m.functions[0].allocations:
        if not isinstance(alloc, mybir.MemoryLocationSet):
            continue
        name = alloc.memorylocations[0].name
        if alloc.kind == "ExternalInput":
            if name != partition_name:
                in_names.append(name)
        elif alloc.kind == "ExternalOutput":
            shape = tuple(alloc.tensor_shape)
            dtype = mybir.dt.np(alloc.dtype)
            out_avals.append(jax.core.ShapedArray(shape, dtype))
            zero_shapes.append((shape, dtype))
            out_names.append(name)
    n_params = len(in_names)
    n_outs = len(out_avals)
    all_names = tuple(in_names + out_names)
    if partition_name is not None:
        all_names = all_names + (partition_name,)
    donate = tuple(range(n_params, n_params + n_outs))

    def _body(*args):
        operands = list(args)
        if partition_name is not None:
            operands.append(partition_id_tensor())
        outs = _bass_exec_p.bind(
            *operands,
            out_avals=tuple(out_avals),
            in_names=all_names,
            out_names=tuple(out_names),
            lowering_input_output_aliases=(),
            sim_require_finite=True,
            sim_require_nnan=True,
            nc=nc,
        )
        return tuple(outs)

    devices = jax.devices()[:NCORES]
    mesh = Mesh(np.asarray(devices), ("core",))
    sharded = jax.jit(
        shard_map(
            _body,
            mesh=mesh,
            in_specs=(PartitionSpec("core"),) * (n_params + n_outs),
            out_specs=(PartitionSpec("core"),) * n_outs,
            check_rep=False,
        ),
        donate_argnums=donate,
        keep_unused=True,
    )

    def run(in_maps):
        concat_in = [
            np.concatenate([m[name] for m in in_maps], axis=0)
            for name in in_names
        ]
        concat_zeros = [
            np.zeros((NCORES * s[0], *s[1:]), dt) for s, dt in zero_shapes
        ]
        out_arrs = sharded(*concat_in, *concat_zeros)
        return [
            {
                name: np.asarray(out_arrs[i]).reshape(
                    NCORES, *out_avals[i].shape
                )[c]
                for i, name in enumerate(out_names)
            }
            for c in range(NCORES)
        ]

    return run


def make_in_maps(x, gw, gb, pw, pb):
    """Shard + lay out the full inputs into per-core input maps."""
    import ml_dtypes
    bf = ml_dtypes.bfloat16
    pwt = np.ascontiguousarray(
        pw.transpose(2, 1, 0).astype(bf))                             # [L, D_IN, D_OUT]
    gwr = np.ascontiguousarray(np.asarray(gw, np.float32).astype(bf))
    gbc = np.ascontiguousarray(gb, dtype=np.float32).reshape(L, 1)
    in_maps = []
    for c in range(NCORES):
        xtc = np.ascontiguousarray(
            np.asarray(x[c * BSH:(c + 1) * BSH, :], np.float32).T.astype(bf))
        in_maps.append({"xt": xtc, "pwt": pwt, "gwt": gwr, "gb": gbc})
    return in_maps


def finish_host(results, pb):
    """Normalize by the gate sum and add the host-side pb bias term."""
    pbf = np.asarray(pb, np.float32)                  # [D_OUT, L]
    outs = []
    for r in results:
        et = r["ets"].astype(np.float32)              # [L, BSH] unnormalized E
        den = et.sum(axis=0)                          # [BSH]
        g = (et / den).T                              # [BSH, L] gatings
        outs.append(r["outt"].T / den[:, None] + g @ pbf.T)
    return np.ascontiguousarray(np.concatenate(outs, axis=0), dtype=np.float32)


def _get_runner():
    global _RUNNER
    if _RUNNER is None:
        nc = _build_module()
        try:
            _RUNNER = _make_runner(nc)
        except Exception:
            # Fallback: the (slower, non-cached) stock execution path.
            from concourse.bass_utils import run_bass_kernel_spmd

            def _run(in_maps):
                return run_bass_kernel_spmd(
                    nc, in_maps, core_ids=list(range(NCORES))
                ).results

            _RUNNER = _run
    return _RUNNER


def kernel(x, gw, gb, pw, pb):
    global _RUNNER
    in_maps = make_in_maps(x, gw, gb, pw, pb)
    try:
        results = _get_runner()(in_maps)
    except Exception:
        # One retry with a freshly built runner (e.g. transient device error).
        _RUNNER = None
        results = _get_runner()(in_maps)
    return finish_host(results, pb)
